# revision 45
# baseline (speedup 1.0000x reference)
"""Trainium2 Bass kernel for DynamicLocalGlobalRouter.

Reference computation (B=2, H=16, S=2048, D=64, radius=16):
  local_out = sliding-window softmax attention (window 33) per (b,h)
  gate      = sigmoid(mean_s(q) @ w_gate + b_gate)      per (b,h)
  out       = gate * local_out + (1-gate) * global_out

Sharding: B*H = 32 (b,h) pairs -> 4 pairs per core across 8 cores.

Device algorithm (per pair), key-stationary banded attention:
  - Host supplies q/k transposed+padded to [64, S+pad] (bf16) with two
    pairs stacked on the partition axis; V is [128, 17 chunks, 64] with
    keys-within-chunk on partitions.  Input DMAs are split into chunk-
    ordered pieces so compute starts ~1us into the (per-iteration) DMA
    stream instead of waiting for whole tensors.
  - Scores per 128-key chunk c: [128 keys, 160-query span] matmuls into
    psum groups of 6 chunks; exp on ACT (one op per group); band-mask
    multiply on DVE (bf16 2x mode).
  - PV: per chunk, main (128-query) + spill (32-query) matmuls with
    rhs = [V chunk | validity] (65 cols); the 65th column accumulates the
    softmax denominator Z (zero rows for out-of-range keys make boundary
    handling exact).  pv psum tiles hold 7/7/2 blocks of 65 fp32 columns
    (one 2KB bank each); within a bank the block's accumulate matmul is
    emitted before the next block's start=True (which clears the bank's
    has_written bits).
  - Gate: sigmoid(w.mean_s(q)+b) for BOTH stacked pairs at once via a
    block-diagonal [128,2] weight lhsT (5 strip matmuls), exp-only ACT
    chain, then two tiny matmuls broadcast (g, 1-g) to 128 partitions.
  - Tail per pv tile: zr=recip(Z cols, strided); zrg=zr*g (DVE 4x);
    wide tensor_tensor mult pv*(zrg broadcast) -> bf16 tmp; bf16 add
    with (1-g)-prescaled global (alternating DVE / GpSimd).  Output is
    written bf16 and cast to fp32 on the host (tolerance 2e-2 >> ulp).
All numerics except matmul inputs and the final output stay fp32.
"""

import os
import sys
from contextlib import ExitStack

import numpy as np

sys.path.insert(0, "/opt/trn_rl_repo")

import bass_rust  # noqa: E402
import concourse.bacc as bacc  # noqa: E402
import concourse.tile as tile  # noqa: E402
from concourse import mybir  # noqa: E402
from concourse.bass_utils import run_bass_kernel_spmd  # noqa: E402

B, H, S, D = 2, 16, 2048, 64
RADIUS = 16
NCORES = 8
PAIRS = B * H            # 32
PPC = PAIRS // NCORES    # 4 pairs per core
NB = S // 128            # 16 query/key blocks
NCH = NB + 1             # 17 key chunks (chunk c covers keys [c*128-16, c*128+112))
SPAN = 160               # query span per key chunk
QT_W = 32 + S + 128      # 2208 padded Q_T width (col i <-> query i-32)
KT_W = 16 + S + 112      # 2176 padded K_T width (col j <-> key j-16)
VS_W = NCH * 65          # 1105: V chunks + ones/validity column
GRP = 6                  # score chunks batched per psum group tile
GRP_W = 1024             # group tile width: 2 psum banks, 3 x 160 slots per bank
GATE_STRIP = 448         # gate matmul strip width
BC = GATE_STRIP          # broadcast gate scalars offset in misc tile (4 cols)
PVT_B = (7, 7, 2)        # blocks per pv psum tile (7*65=455 fp32 <= one bank)
# qk DMA pieces for the cold pair-pair: (kt_lo, kt_hi, qt_lo, qt_hi) per
# score group (group g needs kt cols [g*768, ...) and qt [g*768, g*768+928))
GRP_PIECES_4 = (
    (0, 384, 0, 544),
    (384, 768, 544, 928),
    (768, 1536, 928, 1696),
    (1536, KT_W, 1696, QT_W),
)
GRP_PIECES_3 = (
    (0, 768, 0, 928),
    (768, 1536, 928, 1696),
    (1536, KT_W, 1696, QT_W),
)
GRP_PIECES = GRP_PIECES_3
OUT_SPLIT = True

F32 = mybir.dt.float32
BF16 = mybir.dt.bfloat16

import ml_dtypes  # noqa: E402

NP_BF16 = ml_dtypes.bfloat16

# set by test harness to capture an NTFF profile
TRACE = bool(int(os.environ.get("KERNEL_TRACE", "0")))
LAST_RESULT = None

_CACHE = {}

# bisection knobs for benchmarking: subset of
# {"scores", "exp", "mask", "pv", "z", "tail", "dmain", "gate"}
DISABLE = frozenset()
# debug build: dump pair-0 intermediates to a "dbg" output
DEBUG = False
# tail engine split: mult must stay on DVE (GpSimd cannot read PSUM);
# the bf16 add runs on GpSimd with all-SBUF operands
MULT_POOL = False
# stage lookahead for the PE software pipeline
LOOKAHEAD = 1
# (group, half) mask ops offloaded to GpSimd
MASK_POOL = frozenset()
# use staggered per-engine loop resets (no all-engine barrier per iteration)
STAGGERED = True
# bench copies of the body per loop iteration; >1 lets consecutive kernel
# executions pipeline (no loop branch between them)
BODY_MULT = 1


def _goff(l):
    """free-dim offset of chunk-slot l (0..5) inside a group tile; slots
    avoid straddling the 2KB psum bank boundary (3 x 160 <= 512 per bank)"""
    return (l // 3) * 512 + (l % 3) * 160


def _grp_of(c):
    return min(c // GRP, 2)


def _chunks_of_grp(g):
    return range(g * GRP, min((g + 1) * GRP, NCH))


def _build_program(nc, reps=None):
    qk_d = nc.dram_tensor("qk", [PPC // 2, 128, QT_W + KT_W], BF16, kind="ExternalInput")
    vg_d = nc.dram_tensor("vg", [PPC, 128, VS_W + NB * 64], BF16, kind="ExternalInput")
    # consts: [0:160] band mask, [160:162] block-diag gate weight
    con_d = nc.dram_tensor("con", [128, 162], BF16, kind="ExternalInput")
    # [2, 0:256] row selectors, [2, 256] = -b_gate
    sel_d = nc.dram_tensor("sel", [2, 257], F32, kind="ExternalInput")
    out_d = nc.dram_tensor("out", [PPC, 128, NB * 64], BF16, kind="ExternalOutput")
    if DEBUG:
        dbg_d = nc.dram_tensor("dbg", [4, 128, 1060], F32, kind="ExternalOutput")

    with tile.TileContext(nc) as tc, ExitStack() as ctx:
        consts = ctx.enter_context(tc.tile_pool(name="consts", bufs=1))
        qkp = ctx.enter_context(tc.tile_pool(name="qkp", bufs=2))
        vsp = ctx.enter_context(tc.tile_pool(name="vsp", bufs=3))
        ptp = ctx.enter_context(tc.tile_pool(name="ptp", bufs=3))
        tmpp = ctx.enter_context(tc.tile_pool(name="tmpp", bufs=3))
        outp_p = ctx.enter_context(tc.tile_pool(name="outp", bufs=2))
        smalls = ctx.enter_context(tc.tile_pool(name="smalls", bufs=3))
        ps_s = ctx.enter_context(tc.tile_pool(name="ps_s", bufs=2, space="PSUM"))
        ps_pv = ctx.enter_context(tc.tile_pool(name="ps_pv", bufs=3, space="PSUM"))
        ps_m = ctx.enter_context(tc.tile_pool(name="ps_m", bufs=1, space="PSUM"))

        con_sb = consts.tile([128, 162], BF16, tag="con")
        nc.sync.dma_start(out=con_sb, in_=con_d[:, :])
        mask_sb = con_sb[:, 0:160]
        wg_sb = con_sb[:, 160:162]
        # row-selector lhsTs for gate broadcast + (-b_gate)
        selb_sb = consts.tile([2, 257], F32, tag="sel")
        nc.sync.dma_start(out=selb_sb, in_=sel_d[:, :])
        sel_sb = selb_sb[:, 0:256]
        bgn_sb = selb_sb[:, 256:257]

        dis = DISABLE

        def emit_all():
            # ---- per-pair state ----
            qk_t = [None] * 2          # per pair-pair qk tile
            vs_t = [None] * PPC
            gl_t = [None] * PPC
            pv_t = {}                  # (p, t) -> psum tile [128, PVT_B[t]*65]
            bcs_t = [None] * 2         # per pair-pair gate scalars [128, 4] sbuf
            pt_t = {}                  # (p, g) -> masked prob tile
            st_t = {}                  # (p, g) -> scores psum tile
            outp_t = [None] * PPC
            gls_t = [None] * PPC
            zrg_t = [None] * PPC

            # ---- DMA issue (SP program order = emission order) ----
            def dma_qk(j, pieces):
                t = qkp.tile([128, QT_W + KT_W], BF16, tag="qk")
                qk_t[j] = t
                if "dmain" in dis:
                    return
                if pieces is None:
                    nc.sync.dma_start(out=t, in_=qk_d[j])
                    return
                # piece i covers score-group i's kt/qt column needs
                for klo, khi, qlo, qhi in pieces:
                    nc.sync.dma_start(
                        out=t[:, QT_W + klo : QT_W + khi],
                        in_=qk_d[j, :, QT_W + klo : QT_W + khi],
                    )
                    nc.sync.dma_start(
                        out=t[:, qlo:qhi], in_=qk_d[j, :, qlo:qhi]
                    )

            def dma_vg(p, split):
                t = vsp.tile([128, VS_W + NB * 64], BF16, tag="vg")
                vs_t[p] = t[:, 0:VS_W]
                gl_t[p] = t[:, VS_W:]
                if "dmain" in dis:
                    return
                if split:
                    nc.sync.dma_start(out=t[:, 0:VS_W], in_=vg_d[p, :, 0:VS_W])
                    nc.sync.dma_start(out=t[:, VS_W:], in_=vg_d[p, :, VS_W:])
                else:
                    nc.sync.dma_start(out=t, in_=vg_d[p])

            # chunk-ordered prologue for pair-pair 0; coarse prefetch after
            dma_qk(0, GRP_PIECES)
            dma_vg(0, True)
            dma_vg(1, False)
            dma_qk(1, None)
            dma_vg(2, False)
            dma_vg(3, False)

            def qt(p):
                return qk_t[p // 2][(p % 2) * 64 : (p % 2) * 64 + 64, 0:QT_W]

            def kt(p):
                return qk_t[p // 2][(p % 2) * 64 : (p % 2) * 64 + 64, QT_W:]

            # ---- stage emitters ----
            def emit_S(p, g):
                st = ps_s.tile([128, GRP_W], F32, tag="st", name=f"st{p}_{g}")
                st_t[(p, g)] = st
                if "scores" not in dis:
                    for l, c in enumerate(_chunks_of_grp(g)):
                        off = _goff(l)
                        nc.tensor.matmul(
                            st[:, off : off + SPAN],
                            lhsT=kt(p)[:, c * 128 : (c + 1) * 128],
                            rhs=qt(p)[:, c * 128 : c * 128 + SPAN],
                            start=True,
                            stop=True,
                        )
                pT = ptp.tile([128, 960], BF16, tag="pT", name=f"pT{p}_{g}")
                pt_t[(p, g)] = pT
                nch = len(_chunks_of_grp(g))
                if "exp" not in dis:
                    # exp+mask per half-group (3 psum slots) into a PACKED pT
                    # layout; fine granularity keeps the exp->mask->pv chain
                    # pipelined across engines
                    for h in range(2):
                        nsl = min(3, nch - 3 * h)
                        if nsl <= 0:
                            break
                        nc.scalar.activation(
                            pT[:, h * 480 : h * 480 + nsl * 160],
                            st[:, h * 512 : h * 512 + nsl * 160],
                            mybir.ActivationFunctionType.Exp, scale=1.0 / np.sqrt(D),
                        )
                if "mask" not in dis:
                    for h in range(2):
                        nsl = min(3, nch - 3 * h)
                        if nsl <= 0:
                            break
                        v = pT[:, h * 480 : h * 480 + nsl * 160].rearrange(
                            "p (b w) -> p b w", w=160
                        )
                        m = mask_sb[:, :].unsqueeze(1).broadcast_to((128, nsl, 160))
                        eng = nc.gpsimd if (g, h) in MASK_POOL else nc.vector
                        eng.tensor_tensor(
                            out=v, in0=v, in1=m, op=mybir.AluOpType.mult
                        )

            def _blk_tile(b):
                t = 2 if b >= 14 else b // 7
                return t, b - (14 if t == 2 else 7 * t)

            def emit_tail(p, t):
                """pv tile t (blocks 7t..) is fully accumulated."""
                if "tail" in dis:
                    return
                nb = PVT_B[t]
                b0 = 14 if t == 2 else 7 * t
                pv = pv_t[(p, t)][:, 0 : nb * 65]
                pv3 = pv.rearrange("p (b w) -> p b w", w=65)
                bcs = bcs_t[p // 2]
                g_sc = 1.0 if bcs is None else bcs[:, 2 * (p % 2) : 1 + 2 * (p % 2)]
                mg_sc = 1.0 if bcs is None else bcs[:, 1 + 2 * (p % 2) : 2 + 2 * (p % 2)]
                if t == 0:
                    zrg_t[p] = smalls.tile([128, NB], F32, tag="zrg", name=f"zrg{p}")
                    # (1-g)-prescaled global for the whole pair (DVE 4x mode)
                    gls_t[p] = tmpp.tile(
                        [128, NB * 64], BF16, tag="gls", name=f"gls{p}"
                    )
                    nc.vector.tensor_scalar_mul(gls_t[p], gl_t[p], mg_sc)
                zr = zrg_t[p][:, b0 : b0 + nb]
                nc.vector.reciprocal(zr, pv3[:, :, 64])
                nc.vector.tensor_scalar_mul(zr, zr, g_sc)
                tmp = tmpp.tile([128, nb * 64], BF16, tag="tmp", name=f"tmp{p}_{t}")
                t3 = tmp.rearrange("p (b w) -> p b w", w=64)
                zb = zr.unsqueeze(-1).broadcast_to((128, nb, 64))
                meng = nc.gpsimd if MULT_POOL else nc.vector
                meng.tensor_tensor(
                    out=t3, in0=pv3[:, :, 0:64], in1=zb, op=mybir.AluOpType.mult
                )
                aeng = nc.vector if MULT_POOL else nc.gpsimd
                aeng.tensor_tensor(
                    out=outp_t[p][:, b0 * 64 : (b0 + nb) * 64],
                    in0=tmp,
                    in1=gls_t[p][:, b0 * 64 : (b0 + nb) * 64],
                    op=mybir.AluOpType.add,
                )
                if t == 1 and p == PPC - 1 and OUT_SPLIT:
                    # early drain: ship blocks 0..13 as soon as they're done
                    nc.sync.dma_start(
                        out=out_d[p, :, 0 : 14 * 64], in_=outp_t[p][:, 0 : 14 * 64]
                    )
                if t == 2:
                    if p == PPC - 1 and OUT_SPLIT:
                        nc.sync.dma_start(
                            out=out_d[p, :, 14 * 64 :], in_=outp_t[p][:, 14 * 64 :]
                        )
                    else:
                        nc.sync.dma_start(out=out_d[p], in_=outp_t[p])
                if DEBUG and p == 0:
                    dpv = tmpp.tile([128, nb * 65], F32, tag="dpv", name=f"dpv{t}")
                    nc.vector.tensor_copy(dpv, pv)
                    nc.sync.dma_start(
                        out=dbg_d[1, :, b0 * 65 : (b0 + nb) * 65], in_=dpv
                    )
                    if t == 2:
                        nc.sync.dma_start(out=dbg_d[2, :, 0:16], in_=zrg_t[p])
                    nc.gpsimd.dma_start(
                        out=dbg_d[3, :, b0 * 64 : (b0 + nb) * 64], in_=tmp
                    )

            def emit_P(p, g):
                if g == 0:
                    outp_t[p] = outp_p.tile(
                        [128, NB * 64], BF16, tag="po", name=f"po{p}"
                    )
                pT = pt_t[(p, g)]
                vs3 = vs_t[p].rearrange("p (c w) -> p c w", w=65)
                for l, c in enumerate(_chunks_of_grp(g)):
                    off = l * 160
                    spill = pT[:, off : off + 32]
                    main = pT[:, off + 32 : off + SPAN]
                    if c > 0 and "pv" not in dis:
                        bt, bs = _blk_tile(c - 1)
                        nc.tensor.matmul(
                            pv_t[(p, bt)][96:128, bs * 65 : bs * 65 + 65],
                            lhsT=spill,
                            rhs=vs3[:, c, :],
                            start=False,
                            stop=True,
                            skip_group_check=True,
                            tile_position=(0, 96),
                        )
                    if c in (7, 14):
                        emit_tail(p, c // 7 - 1)
                    if c < NB:
                        bt, bs = _blk_tile(c)
                        if bs == 0:
                            # full-bank tile so pool packing stays bank-aligned
                            pv_t[(p, bt)] = ps_pv.tile(
                                [128, 512], F32, tag="pv", name=f"pv{p}_{bt}"
                            )
                        if "pv" not in dis:
                            nc.tensor.matmul(
                                pv_t[(p, bt)][:, bs * 65 : bs * 65 + 65],
                                lhsT=main,
                                rhs=vs3[:, c, :],
                                start=True,
                                stop=False,
                                skip_group_check=True,
                            )
                if g == 2:
                    emit_tail(p, 2)

            def emit_gate(p):
                """block-diagonal gate for pairs p, p+1"""
                if "gate" in dis:
                    return
                misc = ps_m.tile([128, 512], F32, tag="misc", name=f"m{p}")
                # arm the bank's has_written bits across ALL partitions with a
                # 1-col M=128 start=True matmul; everything after uses
                # start=False (first write per element overwrites, later ones
                # accumulate) - semantics verified on HW
                nc.tensor.matmul(
                    misc[:, 504:505],
                    lhsT=sel_sb[:, 0:128],
                    rhs=bgn_sb,
                    start=True,
                    stop=False,
                    skip_group_check=True,
                )
                for t in range(5):
                    w = GATE_STRIP if t < 4 else S - 4 * GATE_STRIP
                    q0 = 32 + t * GATE_STRIP
                    nc.tensor.matmul(
                        misc[0:2, 0:w],
                        lhsT=wg_sb,
                        rhs=qk_t[p // 2][:, q0 : q0 + w],
                        start=False,
                        stop=(t == 4),
                        skip_group_check=True,
                    )
                scl = smalls.tile([2, 8], F32, tag="scl", name=f"scl{p}")
                nc.vector.reduce_sum(
                    scl[:, 0:1], misc[0:2, 0:GATE_STRIP], axis=mybir.AxisListType.X
                )
                # 1/g = 1 + exp(-(x/S + b));  g = recip;  1-g
                nc.scalar.activation(
                    scl[:, 1:2],
                    scl[:, 0:1],
                    mybir.ActivationFunctionType.Exp,
                    bias=bgn_sb[0:2, 0:1],
                    scale=-1.0 / S,
                )
                nc.vector.tensor_scalar(
                    scl[:, 2:3], scl[:, 1:2], 1.0, None, op0=mybir.AluOpType.add
                )
                nc.vector.reciprocal(scl[:, 3:4], scl[:, 2:3])
                nc.vector.tensor_scalar(
                    scl[:, 4:5],
                    scl[:, 3:4],
                    -1.0,
                    1.0,
                    op0=mybir.AluOpType.mult,
                    op1=mybir.AluOpType.add,
                )
                # broadcast (gA, 1-gA) / (gB, 1-gB) across partitions, then
                # copy to SBUF so the misc bank frees early
                nc.tensor.matmul(
                    misc[:, BC : BC + 2],
                    lhsT=sel_sb[:, 0:128],
                    rhs=scl[:, 3:5],
                    start=False,
                    stop=False,
                    skip_group_check=True,
                )
                nc.tensor.matmul(
                    misc[:, BC + 2 : BC + 4],
                    lhsT=sel_sb[:, 128:256],
                    rhs=scl[:, 3:5],
                    start=False,
                    stop=True,
                    skip_group_check=True,
                )
                bcs = smalls.tile([128, 4], F32, tag="bcs", name=f"bcs{p}")
                nc.vector.tensor_copy(bcs, misc[:, BC : BC + 4])
                bcs_t[p // 2] = bcs

            # ---- stage-interleaved schedule with lookahead 2 ----
            # gate(p) right after P(p,0): its misc bank is armed by Z(0)'s
            # start=True, and S(p,2) has already consumed the full qt so the
            # gate matmuls don't add a DMA stall.
            stages = [(p, g) for p in range(PPC) for g in range(3)]
            LOOK = LOOKAHEAD
            for i in range(len(stages) + LOOK):
                if i < len(stages):
                    emit_S(*stages[i])
                if i >= LOOK:
                    p, g = stages[i - LOOK]
                    emit_P(p, g)
                    if g == 0 and p % 2 == 0:
                        emit_gate(p)

        if reps is None:
            emit_all()
        else:
            engs = [
                mybir.EngineType.PE,
                mybir.EngineType.Activation,
                mybir.EngineType.DVE,
                mybir.EngineType.Pool,
                mybir.EngineType.SP,
            ]
            with tc.For_i(0, reps, 1, hint_engines=engs, staggered_reset=STAGGERED):
                for _ in range(BODY_MULT):
                    emit_all()


def _get_nc(reps=None):
    key = ("nc", reps, DISABLE, BODY_MULT, STAGGERED, MULT_POOL, OUT_SPLIT, LOOKAHEAD, MASK_POOL)
    if key not in _CACHE:
        nc = bacc.Bacc("TRN2", target_bir_lowering=False)
        _build_program(nc, reps=reps)
        nc.compile()
        _CACHE[key] = nc
    return _CACHE[key]


def _band_mask():
    j = np.arange(128)[:, None]
    i = np.arange(SPAN)[None, :]
    return ((j <= i) & (j >= i - 32)).astype(NP_BF16)


def _prepare_in_maps(inputs):
    q = np.ascontiguousarray(np.asarray(inputs["q"], dtype=np.float32))
    k = np.ascontiguousarray(np.asarray(inputs["k"], dtype=np.float32))
    v = np.ascontiguousarray(np.asarray(inputs["v"], dtype=np.float32))
    g = np.ascontiguousarray(np.asarray(inputs["global_out"], dtype=np.float32))
    wgv = np.asarray(inputs["w_gate"], dtype=np.float32).reshape(64)

    qf = q.reshape(PAIRS, S, D)
    kf = k.reshape(PAIRS, S, D)
    vf = v.reshape(PAIRS, S, D)
    gf = g.reshape(PAIRS, S, D)

    # host-side layout marshalling (transpose/pad/shift only, no math);
    # qt/kt stack pair 2j on partitions 0:64 and pair 2j+1 on 64:128
    qk = np.zeros((PAIRS // 2, 128, QT_W + KT_W), NP_BF16)
    qk[:, 0:64, 32 : 32 + S] = qf[0::2].transpose(0, 2, 1)
    qk[:, 64:128, 32 : 32 + S] = qf[1::2].transpose(0, 2, 1)
    qk[:, 0:64, QT_W + 16 : QT_W + 16 + S] = kf[0::2].transpose(0, 2, 1)
    qk[:, 64:128, QT_W + 16 : QT_W + 16 + S] = kf[1::2].transpose(0, 2, 1)

    vs = np.zeros((PAIRS, NCH * 128, 65), NP_BF16)
    vs[:, 16 : 16 + S, 0:64] = vf
    vs[:, 16 : 16 + S, 64] = 1.0
    vs = (
        vs.reshape(PAIRS, NCH, 128, 65)
        .transpose(0, 2, 1, 3)
        .reshape(PAIRS, 128, VS_W)
    )

    gl = gf.reshape(PAIRS, NB, 128, 64).transpose(0, 2, 1, 3).reshape(PAIRS, 128, NB * 64)
    vg = np.ascontiguousarray(
        np.concatenate([vs, gl.astype(NP_BF16)], axis=2)
    )

    con = np.zeros((128, 162), NP_BF16)
    con[:, 0:160] = _band_mask()
    con[0:64, 160] = wgv
    con[64:128, 161] = wgv

    sel = np.zeros((2, 257), np.float32)
    sel[0, 0:128] = 1.0
    sel[1, 128:256] = 1.0
    sel[:, 256] = -float(np.asarray(inputs["b_gate"], dtype=np.float32).reshape(1)[0])

    in_maps = []
    for core in range(NCORES):
        lo, hi = core * PPC, (core + 1) * PPC
        glo, ghi = core * (PPC // 2), (core + 1) * (PPC // 2)
        in_maps.append(
            {
                "qk": np.ascontiguousarray(qk[glo:ghi]),
                "vg": vg[lo:hi],
                "con": con,
                "sel": sel,
            }
        )
    return in_maps


def _unshard_out(results):
    outs = np.stack([results[i]["out"] for i in range(NCORES)])  # [8,4,128,NB*64]
    out = (
        outs.astype(np.float32)
        .reshape(PAIRS, 128, NB, 64)
        .transpose(0, 2, 1, 3)
        .reshape(B, H, S, D)
    )
    return np.ascontiguousarray(out)


def kernel(**inputs):
    global LAST_RESULT
    in_maps = _prepare_in_maps(inputs)
    nc = _get_nc()
    try:
        res = run_bass_kernel_spmd(
            nc, in_maps, core_ids=list(range(NCORES)), trace=TRACE
        )
    except ModuleNotFoundError:
        # NTFF profiling hook unavailable in this axon build
        res = run_bass_kernel_spmd(
            nc, in_maps, core_ids=list(range(NCORES)), trace=False
        )
    LAST_RESULT = res
    return _unshard_out(res.results)


def bench_hw_ns(inputs, reps_lo=16, reps_hi=4112, runs=16):
    """Estimate per-invocation HW time via in-NEFF repetition.

    Runs the same program with the body looped reps_lo and reps_hi times;
    the wall-clock delta divided by the rep delta isolates on-device time
    from compile/shipping/dispatch overhead.
    """
    import time

    in_maps = _prepare_in_maps(inputs)

    def run_variant(reps):
        nc = _get_nc(reps=reps)
        times = []
        for r in range(runs + 1):
            t0 = time.time()
            run_bass_kernel_spmd(nc, in_maps, core_ids=list(range(NCORES)))
            t1 = time.time()
            if r > 0:  # first run includes NEFF compile
                times.append(t1 - t0)
        return min(times)

    t_lo = run_variant(reps_lo)
    t_hi = run_variant(reps_hi)
    # delta counts (reps_hi-reps_lo) loop iterations x BODY_MULT kernel
    # executions each; report steady-state time per kernel execution
    per_iter_ns = (t_hi - t_lo) / ((reps_hi - reps_lo) * BODY_MULT) * 1e9
    return per_iter_ns, t_lo, t_hi


if __name__ == "__main__":
    rng = np.random.default_rng(0)
    ins = {
        "q": rng.standard_normal((B, H, S, D), dtype=np.float32),
        "k": rng.standard_normal((B, H, S, D), dtype=np.float32),
        "v": rng.standard_normal((B, H, S, D), dtype=np.float32),
        "global_out": rng.standard_normal((B, H, S, D), dtype=np.float32),
        "buckets": rng.integers(0, 64, size=(B, S)),
        "w_gate": rng.standard_normal(64, dtype=np.float32) / 8.0,
        "b_gate": np.zeros(1, np.float32),
    }
    out = kernel(**ins)
    print("out", out.shape, out.dtype, float(np.abs(out).max()))
